# revision 1
# baseline (speedup 1.0000x reference)
"""Trainium2 Bass kernel for nn_Net_LSTM_cell (4-direction LSTM over features).

Model (B=4096, IN=4096, FS=4096, S=64, D=64, H=512):
  feat = relu(x @ W1.T + b1)                       # (B, FS)
  4 LSTM cells (left/right/up/down; up+down share the "down" weights) scanned
  for S=64 steps over per-step (B, 64) feature slices; final hidden states
  concat -> (B, 2048) -> W3 -> log_softmax -> (B, 10).

Sharding: pure data-parallel over batch across 8 NeuronCores (B=512/core),
weights replicated, zero collectives. Host transposes/pads/casts inputs; the
output is gathered by simple concatenation.

On-device layout is fully transposed ("T" = feature-on-partitions):
  featT [FS, B] in DRAM scratch; per-step x slices are contiguous row-slabs
  (left/right) or stride-64 row gathers (up/down).
  Recurrent state hT/cT [128p, 4kc, 512b]; gates computed as
  gT [gate-dim on partitions, batch free] so LSTM biases are per-partition
  scalars fused into the sigmoid/tanh activations for free.
  Matmul operands are fp16 (1 cycle/row on the PE; PSUM accumulates fp32);
  all elementwise math + c-state stay fp32.

The left+right (and up+down) per-step inputs are packed into one [128, 512]
rhs tile; each cell's Wih.T is zero-padded on the host to K=128 so the
"other" half contributes nothing.
"""

import numpy as np

import concourse.bacc as bacc
import concourse.mybir as mybir
import concourse.tile as tile
from concourse import bass_utils

# ---- problem dims (hardcoded per contract) ----
B_FULL, IN, FS, S, H = 4096, 4096, 4096, 64, 512
NCORES = 8
B = B_FULL // NCORES          # 512 per core
GH = 4 * H                    # 2048 gate dim
P = 128
KH = H // P                   # 4 hidden-dim chunks
KIN = IN // P                 # 32
MFS = FS // P                 # 32
NBT = B // P                  # 4 batch tiles (epilogue)

F32 = mybir.dt.float32
F16 = mybir.dt.float16
AF = mybir.ActivationFunctionType

_CACHE = {}


def _emit(nc, tc, t):
    from contextlib import ExitStack
    with ExitStack() as ctx:
        dram = ctx.enter_context(tc.tile_pool(name="dram", bufs=1, space="DRAM"))
        wb = ctx.enter_context(tc.tile_pool(name="wb", bufs=1))

        feat = dram.tile([FS, B], F16, name="featT")

        # ---- persistent weights + state (DMAs overlap with layer 1) ----
        whh_sb = []
        for i in range(3):
            w_ = wb.tile([P, KH, GH], F16, name=f"whh{i}", tag=f"whh{i}")
            nc.sync.dma_start(w_[:], t["whh"].ap()[i])
            whh_sb.append(w_)
        wih_sb = []
        for j in range(4):
            w_ = wb.tile([P, GH], F16, name=f"wih{j}", tag=f"wih{j}")
            nc.sync.dma_start(w_[:], t["wih"].ap()[j])
            wih_sb.append(w_)
        bg_sb = []
        for i in range(3):
            b_ = wb.tile([P, 16], F32, name=f"bg{i}", tag=f"bg{i}")
            nc.sync.dma_start(b_[:], t["bgt"].ap()[i])
            bg_sb.append(b_)
        h_sb, c_sb = [], []
        for j in range(4):
            h_ = wb.tile([P, KH, B], F16, name=f"h{j}", tag=f"h{j}")
            nc.sync.dma_start(h_[:], t["h0t"].ap()[j])
            h_sb.append(h_)
            c_ = wb.tile([P, KH, B], F32, name=f"c{j}", tag=f"c{j}")
            nc.sync.dma_start(c_[:], t["c0t"].ap()[j])
            c_sb.append(c_)
        w3_sb = wb.tile([P, 16, 10], F16, name="w3_sb")
        nc.sync.dma_start(w3_sb[:], t["w3t"].ap())
        b3_sb = wb.tile([1, 10], F16, name="b3_sb")
        nc.sync.dma_start(b3_sb[:], t["b3t"].ap())
        ones_sb = wb.tile([1, P], F16, name="ones_sb")
        nc.vector.memset(ones_sb[:], 1.0)

        # ---- phase A: featT = relu(W1 @ xT + b1) -> DRAM scratch ----
        with tc.tile_pool(name="l1w", bufs=3) as l1w, \
             tc.tile_pool(name="l1x", bufs=1) as l1x, \
             tc.tile_pool(name="l1o", bufs=4) as l1o, \
             tc.tile_pool(name="ps1", bufs=4, space="PSUM") as ps1:
            b1_sb = l1x.tile([P, MFS], F32, name="b1_sb")
            nc.sync.dma_start(b1_sb[:], t["b1t"].ap())
            xt_sb = l1x.tile([P, KIN, B], F16, name="xt_sb")
            nc.sync.dma_start(xt_sb[:, :16, :], t["xt"].ap()[:, :16, :])
            nc.sync.dma_start(xt_sb[:, 16:, :], t["xt"].ap()[:, 16:, :])
            for mc in range(MFS):
                w1_sb = l1w.tile([P, KIN, P], F16, name="w1_sb", tag="w1_sb")
                nc.sync.dma_start(w1_sb[:], t["w1t"].ap()[mc])
                ps = ps1.tile([P, B], F32, name="l1_ps", tag="l1_ps")
                for kc in range(KIN):
                    nc.tensor.matmul(ps[:], lhsT=w1_sb[:, kc, :],
                                     rhs=xt_sb[:, kc, :],
                                     start=(kc == 0), stop=(kc == KIN - 1))
                fo = l1o.tile([P, B], F16, name="fo", tag="fo")
                nc.scalar.activation(fo[:], ps[:], AF.Relu,
                                     bias=b1_sb[:, mc:mc + 1])
                nc.sync.dma_start(feat[mc * P:(mc + 1) * P, :], fo[:])

        # row r of V_l[t] is feat row t*64+r (left input at step t);
        # row s of V_u[t] is feat row s*64+t (up input at step t)
        V_l = feat.rearrange("(t r) b -> t r b", r=S)
        V_u = feat.rearrange("(s t) b -> t s b", t=S)

        # cells: 0=left, 1=right, 2=up, 3=down (up/down share weight set 2)
        cell_w = [(whh_sb[0], wih_sb[0], bg_sb[0]),
                  (whh_sb[1], wih_sb[1], bg_sb[1]),
                  (whh_sb[2], wih_sb[2], bg_sb[2]),
                  (whh_sb[2], wih_sb[3], bg_sb[2])]

        # ---- phase B: 64 recurrence steps ----
        with tc.tile_pool(name="xs", bufs=3) as xs, \
             tc.tile_pool(name="tmp", bufs=3) as tmp, \
             tc.tile_pool(name="ps2", bufs=2, space="PSUM") as ps2:
            for st in range(S):
                x_lr = xs.tile([P, B], F16, name="x_lr", tag="x_lr")
                nc.sync.dma_start(x_lr[0:64, :], V_l[st])
                nc.sync.dma_start(x_lr[64:128, :], V_l[S - 1 - st])
                x_ud = xs.tile([P, B], F16, name="x_ud", tag="x_ud")
                nc.sync.dma_start(x_ud[0:64, :], V_u[st])
                nc.sync.dma_start(x_ud[64:128, :], V_u[S - 1 - st])
                xrhs = [x_lr, x_lr, x_ud, x_ud]
                for j in range(4):
                    whh_j, wih_j, bg_j = cell_w[j]
                    h_j, c_j, x_j = h_sb[j], c_sb[j], xrhs[j]
                    for q in range(KH):
                        ps = ps2.tile([P, 4, 512], F32, name="gps", tag="gps")
                        for g in range(4):
                            moff = g * 512 + q * 128
                            nc.tensor.matmul(ps[:, g, :],
                                             lhsT=wih_j[:, moff:moff + P],
                                             rhs=x_j[:],
                                             start=True, stop=False)
                            for kc in range(KH):
                                nc.tensor.matmul(ps[:, g, :],
                                                 lhsT=whh_j[:, kc, moff:moff + P],
                                                 rhs=h_j[:, kc, :],
                                                 start=False, stop=(kc == KH - 1))
                        ti = tmp.tile([P, B], F32, name="ti", tag="ti")
                        tf = tmp.tile([P, B], F32, name="tf", tag="tf")
                        tg = tmp.tile([P, B], F32, name="tg", tag="tg")
                        to = tmp.tile([P, B], F32, name="to", tag="to")
                        nc.scalar.activation(ti[:], ps[:, 0, :], AF.Sigmoid,
                                             bias=bg_j[:, q:q + 1])
                        nc.scalar.activation(tf[:], ps[:, 1, :], AF.Sigmoid,
                                             bias=bg_j[:, 4 + q:5 + q])
                        nc.scalar.activation(tg[:], ps[:, 2, :], AF.Tanh,
                                             bias=bg_j[:, 8 + q:9 + q])
                        nc.scalar.activation(to[:], ps[:, 3, :], AF.Sigmoid,
                                             bias=bg_j[:, 12 + q:13 + q])
                        t2 = tmp.tile([P, B], F32, name="t2", tag="t2")
                        nc.vector.tensor_mul(t2[:], ti[:], tg[:])
                        nc.vector.tensor_mul(c_j[:, q, :], tf[:], c_j[:, q, :])
                        nc.vector.tensor_add(c_j[:, q, :], c_j[:, q, :], t2[:])
                        tct = tmp.tile([P, B], F32, name="tct", tag="tct")
                        nc.scalar.activation(tct[:], c_j[:, q, :], AF.Tanh)
                        nc.vector.tensor_mul(h_j[:, q, :], to[:], tct[:])

            # ---- phase C: logits + log_softmax (inside pools: reuses gps) ----
            for bt in range(NBT):
                lps = ps2.tile([P, 10], F32, name="lps", tag="gps")
                for j in range(4):
                    for kc in range(KH):
                        nc.tensor.matmul(
                            lps[:],
                            lhsT=h_sb[j][:, kc, bt * P:(bt + 1) * P],
                            rhs=w3_sb[:, j * 4 + kc, :],
                            start=(j == 0 and kc == 0), stop=False)
                nc.tensor.matmul(lps[:], lhsT=ones_sb[:], rhs=b3_sb[:],
                                 start=False, stop=True)
                mx = tmp.tile([P, 1], F32, name="mx", tag="mx")
                nc.vector.tensor_reduce(mx[:], lps[:],
                                        axis=mybir.AxisListType.X,
                                        op=mybir.AluOpType.max)
                tt = tmp.tile([P, 10], F32, name="tt", tag="tt")
                nc.vector.tensor_single_scalar(tt[:], lps[:], mx[:],
                                               mybir.AluOpType.subtract)
                ex = tmp.tile([P, 10], F32, name="ex", tag="ex")
                se = tmp.tile([P, 1], F32, name="se", tag="se")
                nc.scalar.activation(ex[:], tt[:], AF.Exp, accum_out=se[:])
                ls = tmp.tile([P, 1], F32, name="ls", tag="ls")
                nc.scalar.activation(ls[:], se[:], AF.Ln)
                lp = tmp.tile([P, 10], F32, name="lp", tag="lp")
                nc.vector.tensor_single_scalar(lp[:], tt[:], ls[:],
                                               mybir.AluOpType.subtract)
                nc.sync.dma_start(t["out"].ap()[bt * P:(bt + 1) * P, :], lp[:])


def build():
    if "nc" in _CACHE:
        return _CACHE["nc"]
    nc = bacc.Bacc("TRN2", target_bir_lowering=False, debug=False,
                   enable_asserts=False, num_devices=NCORES)
    t = {
        "xt": nc.dram_tensor("xt", (P, KIN, B), F16, kind="ExternalInput"),
        "w1t": nc.dram_tensor("w1t", (MFS, P, KIN, P), F16, kind="ExternalInput"),
        "b1t": nc.dram_tensor("b1t", (P, MFS), F32, kind="ExternalInput"),
        "whh": nc.dram_tensor("whh", (3, P, KH, GH), F16, kind="ExternalInput"),
        "wih": nc.dram_tensor("wih", (4, P, GH), F16, kind="ExternalInput"),
        "bgt": nc.dram_tensor("bgt", (3, P, 16), F32, kind="ExternalInput"),
        "h0t": nc.dram_tensor("h0t", (4, P, KH, B), F16, kind="ExternalInput"),
        "c0t": nc.dram_tensor("c0t", (4, P, KH, B), F32, kind="ExternalInput"),
        "w3t": nc.dram_tensor("w3t", (P, 16, 10), F16, kind="ExternalInput"),
        "b3t": nc.dram_tensor("b3t", (1, 10), F16, kind="ExternalInput"),
        "out": nc.dram_tensor("out", (B, 10), F32, kind="ExternalOutput"),
    }
    with tile.TileContext(nc) as tc:
        _emit(nc, tc, t)
    nc.compile()
    _CACHE["nc"] = nc
    return nc


def _to_gate_bias(bih, bhh):
    # [p, g*4+q] with gate-dim index g*512 + q*128 + p
    b = (np.asarray(bih, np.float32) + np.asarray(bhh, np.float32))
    return np.ascontiguousarray(
        b.reshape(4, 4, P).transpose(2, 0, 1).reshape(P, 16))


def _hidT(a):
    # (B=512, H=512) slice -> [p, kc, b] with hidden index kc*128+p
    return np.ascontiguousarray(
        np.asarray(a).T.reshape(KH, P, B).transpose(1, 0, 2))


def _prep(inputs):
    i = {k: np.asarray(v) for k, v in inputs.items()}
    f32 = np.float32
    f16 = np.float16

    # shared (replicated) tensors
    w1t = np.ascontiguousarray(
        i["W1"].astype(f16).reshape(MFS, P, KIN, P).transpose(0, 3, 2, 1))
    b1t = np.ascontiguousarray(i["b1"].astype(f32).reshape(MFS, P).T)
    whh = np.stack([
        np.ascontiguousarray(
            i[f"Whh_{s}"].astype(f16).T.reshape(KH, P, GH).transpose(1, 0, 2))
        for s in ("l", "r", "d")])
    z = np.zeros((64, GH), f16)
    wih = np.stack([
        np.concatenate([i["Wih_l"].astype(f16).T, z], axis=0),
        np.concatenate([z, i["Wih_r"].astype(f16).T], axis=0),
        np.concatenate([i["Wih_d"].astype(f16).T, z], axis=0),
        np.concatenate([z, i["Wih_d"].astype(f16).T], axis=0)])
    bgt = np.stack([_to_gate_bias(i[f"bih_{s}"], i[f"bhh_{s}"])
                    for s in ("l", "r", "d")])
    w3t = np.ascontiguousarray(
        i["W3"].astype(f16).T.reshape(16, P, 10).transpose(1, 0, 2))
    b3t = i["b3"].astype(f16).reshape(1, 10)

    in_maps = []
    for c in range(NCORES):
        bs = slice(c * B, (c + 1) * B)
        xt = np.ascontiguousarray(
            i["x"][bs].astype(f16).T.reshape(KIN, P, B).transpose(1, 0, 2))
        h0t = np.stack([_hidT(i["h0"][j, bs].astype(f16)) for j in range(4)])
        c0t = np.stack([_hidT(i["c0"][j, bs].astype(f32)) for j in range(4)])
        in_maps.append({
            "xt": xt, "w1t": w1t, "b1t": b1t, "whh": whh, "wih": wih,
            "bgt": bgt, "h0t": h0t, "c0t": c0t, "w3t": w3t, "b3t": b3t,
        })
    return in_maps


def kernel(**inputs) -> np.ndarray:
    nc = build()
    in_maps = _prep(inputs)
    res = bass_utils.run_bass_kernel_spmd(
        nc, in_maps, core_ids=list(range(NCORES)), trace=False)
    return np.concatenate(
        [res.results[c]["out"] for c in range(NCORES)], axis=0)


# revision 18
# speedup vs baseline: 1.7700x; 1.7700x over previous
"""Trainium2 Bass kernel for nn_Net_LSTM_cell (4-direction LSTM over features).

Model (B=4096, IN=4096, FS=4096, S=64, D=64, H=512):
  feat = relu(x @ W1.T + b1)                       # (B, FS)
  4 LSTM cells (left/right/up/down; up+down share the "down" weights) scanned
  for S=64 steps over per-step (B, 64) feature slices; final hidden states
  concat -> (B, 2048) -> W3 -> log_softmax -> (B, 10).

Sharding: pure data-parallel over batch across 8 NeuronCores (B=512/core),
weights replicated, zero collectives. Host transposes/pads/casts inputs; the
output is gathered by simple concatenation.

On-device layout is fully transposed ("T" = feature-on-partitions):
  featT [FS, B] in DRAM scratch; per-step x slices are contiguous row-slabs
  (left/right) or stride-64 row gathers (up/down).
  Recurrent state hT/cT [128p, 4kc, 512b]; gates computed as
  gT [gate-dim on partitions, batch free] so LSTM biases are per-partition
  scalars fused into the sigmoid/tanh activations for free.
  Matmul operands are fp16 (1 cycle/row on the PE; PSUM accumulates fp32);
  all elementwise math + c-state stay fp32.

The left+right (and up+down) per-step inputs are packed into one [128, 512]
rhs tile; each cell's Wih.T is zero-padded on the host to K=128 so the
"other" half contributes nothing.
"""

import numpy as np

import concourse.bacc as bacc
import concourse.mybir as mybir
import concourse.tile as tile
from concourse import bass_utils

# ---- problem dims (hardcoded per contract) ----
B_FULL, IN, FS, S, H = 4096, 4096, 4096, 64, 512
NCORES = 8
B = B_FULL // NCORES          # 512 per core
GH = 4 * H                    # 2048 gate dim
P = 128
KH = H // P                   # 4 hidden-dim chunks
KIN = IN // P                 # 32
MFS = FS // P                 # 32
NBT = B // P                  # 4 batch tiles (epilogue)

F32 = mybir.dt.float32
F16 = mybir.dt.float16
AF = mybir.ActivationFunctionType

_CACHE = {}


def _emit(nc, tc, t):
    from contextlib import ExitStack
    with ExitStack() as ctx:
        dram = ctx.enter_context(tc.tile_pool(name="dram", bufs=1, space="DRAM"))
        wb = ctx.enter_context(tc.tile_pool(name="wb", bufs=1))

        feat = dram.tile([FS, B], F16, name="featT")

        # ---- persistent weights + state (DMAs emitted after layer-1 input
        # DMAs so phase A starts immediately; they land during layer 1) ----
        whh_sb = [wb.tile([P, KH, GH], F16, name=f"whh{i}", tag=f"whh{i}")
                  for i in range(3)]
        wih_sb = [wb.tile([P, GH], F16, name=f"wih{j}", tag=f"wih{j}")
                  for j in range(3)]
        h_sb = [wb.tile([P, KH, B], F16, name=f"h{j}", tag=f"h{j}")
                for j in range(4)]
        c_sb = [wb.tile([P, KH, B], F32, name=f"c{j}", tag=f"c{j}")
                for j in range(4)]
        w3_sb = wb.tile([P, 16, 10], F16, name="w3_sb")
        b3_sb = wb.tile([1, 10], F16, name="b3_sb")
        ones_sb = wb.tile([1, P], F16, name="ones_sb")

        def _load_persistent():
            for i in range(3):
                nc.sync.dma_start(whh_sb[i][:], t["whh"].ap()[i])
                nc.sync.dma_start(wih_sb[i][:], t["wih"].ap()[i])
            for j in range(4):
                nc.sync.dma_start(h_sb[j][:], t["h0t"].ap()[j])
                nc.sync.dma_start(c_sb[j][:], t["c0t"].ap()[j])
            nc.sync.dma_start(w3_sb[:], t["w3t"].ap())
            nc.sync.dma_start(b3_sb[:], t["b3t"].ap())
            nc.vector.memset(ones_sb[:], 1.0)

        # ---- phase A: featT = relu(W1 @ xT + b1) -> DRAM scratch ----
        with tc.tile_pool(name="l1w", bufs=3) as l1w, \
             tc.tile_pool(name="l1x", bufs=1) as l1x, \
             tc.tile_pool(name="l1o", bufs=4) as l1o, \
             tc.tile_pool(name="ps1", bufs=4, space="PSUM") as ps1:
            b1_sb = l1x.tile([P, MFS], F32, name="b1_sb")
            nc.sync.dma_start(b1_sb[:], t["b1t"].ap())
            xt_sb = l1x.tile([P, KIN, B], F16, name="xt_sb")
            nc.sync.dma_start(xt_sb[:, :16, :], t["xt"].ap()[:, :16, :])
            nc.sync.dma_start(xt_sb[:, 16:, :], t["xt"].ap()[:, 16:, :])
            for mc in range(MFS):
                w1_sb = l1w.tile([P, KIN, P], F16, name="w1_sb", tag="w1_sb")
                nc.sync.dma_start(w1_sb[:], t["w1t"].ap()[mc])
                ps = ps1.tile([P, B], F32, name="l1_ps", tag="l1_ps")
                for kc in range(KIN):
                    nc.tensor.matmul(ps[:], lhsT=w1_sb[:, kc, :],
                                     rhs=xt_sb[:, kc, :],
                                     start=(kc == 0), stop=(kc == KIN - 1))
                fo = l1o.tile([P, B], F16, name="fo", tag="fo")
                nc.scalar.activation(fo[:], ps[:], AF.Relu,
                                     bias=b1_sb[:, mc:mc + 1])
                nc.sync.dma_start(feat[mc * P:(mc + 1) * P, :], fo[:])
                if mc == 3:
                    # enough layer-1 DMA is in flight; queue the recurrence
                    # weights + initial state now so they land during phase A
                    _load_persistent()

        # row r of V_l[t] is feat row t*64+r (left input at step t);
        # row s of V_u[t] is feat row s*64+t (up input at step t)
        V_l = feat.rearrange("(t r) b -> t r b", r=S)
        V_u = feat.rearrange("(s t) b -> t s b", t=S)

        # cells: 0=left, 1=right, 2=up, 3=down (up/down share weight set 2)
        cell_w = [(whh_sb[0], wih_sb[0]), (whh_sb[1], wih_sb[1]),
                  (whh_sb[2], wih_sb[2]), (whh_sb[2], wih_sb[2])]

        # ---- phase B: 64 recurrence steps ----
        # Gate-major psum: one [128, 4x512] psum tile holds a single gate for
        # all 4 hidden chunks, so each sigma/tanh is ONE [128, 2048] ACT op.
        # The LSTM bias rides row 64 of the augmented K=65 x-matmul (x row 64
        # is constant 1.0), so no per-quadruple bias APs are needed.
        with tc.tile_pool(name="xs", bufs=3) as xs, \
             tc.tile_pool(name="tmp", bufs=2) as tmp, \
             tc.tile_pool(name="ps2", bufs=2, space="PSUM") as ps2:
            tails = []  # delayed per-cell tanh(c)+h-mul, emitted one cell late

            def _emit_cell(j, x_j):
                nonlocal tails
                whh_j, wih_j = cell_w[j]
                h_j, c_j = h_sb[j], c_sb[j]

                def _gate_mms(g):
                    ps = ps2.tile([P, KH, 512], F32, name="gps", tag="gps")
                    for q in range(KH):
                        moff = g * 512 + q * 128
                        nc.tensor.matmul(ps[:, q, :],
                                         lhsT=wih_j[0:65, moff:moff + P],
                                         rhs=x_j[0:65, :],
                                         start=True, stop=False)
                        for kc in range(KH):
                            nc.tensor.matmul(ps[:, q, :],
                                             lhsT=whh_j[:, kc, moff:moff + P],
                                             rhs=h_j[:, kc, :],
                                             start=False, stop=(kc == KH - 1))
                    return ps

                ti = tmp.tile([P, KH, B], F32, name="ti", tag="ti", bufs=1)
                tf = tmp.tile([P, KH, B], F32, name="tf", tag="tf", bufs=1)
                tg = tmp.tile([P, KH, B], F32, name="tg", tag="tg", bufs=1)
                to_all = tmp.tile([P, KH, B], F32, name="to_all", tag="to_all")
                ps_i = _gate_mms(0)
                nc.scalar.activation(ti[:], ps_i[:, :, :], AF.Sigmoid)
                ps_g = _gate_mms(2)
                nc.scalar.activation(tg[:], ps_g[:, :, :], AF.Tanh)
                nc.vector.tensor_mul(ti[:], ti[:], tg[:])
                ps_f = _gate_mms(1)
                nc.scalar.activation(tf[:], ps_f[:, :, :], AF.Sigmoid)
                nc.vector.tensor_mul(c_j[:, :, :], tf[:], c_j[:, :, :])
                nc.vector.tensor_add(c_j[:, :, :], c_j[:, :, :], ti[:])
                ps_o = _gate_mms(3)
                nc.scalar.activation(to_all[:], ps_o[:, :, :], AF.Sigmoid)
                for f in tails:
                    f()
                tails = []

                def _tail():
                    tct = tmp.tile([P, KH, B], F32, name="tct", tag="tct")
                    nc.scalar.activation(tct[:], c_j[:, :, :], AF.Tanh)
                    nc.vector.tensor_mul(h_j[:, :, :], to_all[:], tct[:])
                tails = [_tail]

            def _x_tile(tag, src):
                x_ = xs.tile([P, B], F16, name=tag, tag=tag)
                nc.sync.dma_start(x_[0:64, :], src)
                nc.vector.memset(x_[64:65, :], 1.0)
                return x_

            import os
            n_steps = int(os.environ.get("LSTM_STEPS", str(S)))
            for st in range(n_steps):
                x_l = _x_tile("x_l", V_l[st])
                x_r = _x_tile("x_r", V_l[S - 1 - st])
                x_u = _x_tile("x_u", V_u[st])
                x_d = _x_tile("x_d", V_u[S - 1 - st])
                for j, x_j in enumerate((x_l, x_r, x_u, x_d)):
                    _emit_cell(j, x_j)
            for f in tails:
                f()

            # ---- phase C: logits + log_softmax (inside pools: reuses gps) ----
            for bt in range(NBT):
                lps = ps2.tile([P, 10], F32, name="lps", tag="gps")
                for j in range(4):
                    for kc in range(KH):
                        nc.tensor.matmul(
                            lps[:],
                            lhsT=h_sb[j][:, kc, bt * P:(bt + 1) * P],
                            rhs=w3_sb[:, j * 4 + kc, :],
                            start=(j == 0 and kc == 0), stop=False)
                nc.tensor.matmul(lps[:], lhsT=ones_sb[:], rhs=b3_sb[:],
                                 start=False, stop=True)
                mx = tmp.tile([P, 1], F32, name="mx", tag="mx")
                nc.vector.tensor_reduce(mx[:], lps[:],
                                        axis=mybir.AxisListType.X,
                                        op=mybir.AluOpType.max)
                tt = tmp.tile([P, 10], F32, name="tt", tag="tt")
                nc.vector.tensor_single_scalar(tt[:], lps[:], mx[:],
                                               mybir.AluOpType.subtract)
                ex = tmp.tile([P, 10], F32, name="ex", tag="ex")
                se = tmp.tile([P, 1], F32, name="se", tag="se")
                nc.scalar.activation(ex[:], tt[:], AF.Exp, accum_out=se[:])
                ls = tmp.tile([P, 1], F32, name="ls", tag="ls")
                nc.scalar.activation(ls[:], se[:], AF.Ln)
                lp = tmp.tile([P, 10], F32, name="lp", tag="lp")
                nc.vector.tensor_single_scalar(lp[:], tt[:], ls[:],
                                               mybir.AluOpType.subtract)
                nc.sync.dma_start(t["out"].ap()[bt * P:(bt + 1) * P, :], lp[:])


def build():
    if "nc" in _CACHE:
        return _CACHE["nc"]
    nc = bacc.Bacc("TRN2", target_bir_lowering=False, debug=False,
                   enable_asserts=False, num_devices=NCORES)
    t = {
        "xt": nc.dram_tensor("xt", (P, KIN, B), F16, kind="ExternalInput"),
        "w1t": nc.dram_tensor("w1t", (MFS, P, KIN, P), F16, kind="ExternalInput"),
        "b1t": nc.dram_tensor("b1t", (P, MFS), F32, kind="ExternalInput"),
        "whh": nc.dram_tensor("whh", (3, P, KH, GH), F16, kind="ExternalInput"),
        "wih": nc.dram_tensor("wih", (3, P, GH), F16, kind="ExternalInput"),
        "h0t": nc.dram_tensor("h0t", (4, P, KH, B), F16, kind="ExternalInput"),
        "c0t": nc.dram_tensor("c0t", (4, P, KH, B), F32, kind="ExternalInput"),
        "w3t": nc.dram_tensor("w3t", (P, 16, 10), F16, kind="ExternalInput"),
        "b3t": nc.dram_tensor("b3t", (1, 10), F16, kind="ExternalInput"),
        "out": nc.dram_tensor("out", (B, 10), F32, kind="ExternalOutput"),
    }
    with tile.TileContext(nc) as tc:
        _emit(nc, tc, t)
    nc.compile()
    _CACHE["nc"] = nc
    return nc


def _hidT(a):
    # (B=512, H=512) slice -> [p, kc, b] with hidden index kc*128+p
    return np.ascontiguousarray(
        np.asarray(a).T.reshape(KH, P, B).transpose(1, 0, 2))


def _prep(inputs):
    i = {k: np.asarray(v) for k, v in inputs.items()}
    f32 = np.float32
    f16 = np.float16

    # shared (replicated) tensors
    w1t = np.ascontiguousarray(
        i["W1"].astype(f16).reshape(MFS, P, KIN, P).transpose(0, 3, 2, 1))
    b1t = np.ascontiguousarray(i["b1"].astype(f32).reshape(MFS, P).T)
    whh = np.stack([
        np.ascontiguousarray(
            i[f"Whh_{s}"].astype(f16).T.reshape(KH, P, GH).transpose(1, 0, 2))
        for s in ("l", "r", "d")])
    # wih rows: 0-63 Wih.T, row 64 = combined bias (x row 64 is 1.0), rest 0
    def _wih_aug(s):
        w = np.zeros((P, GH), f16)
        w[0:64] = i[f"Wih_{s}"].astype(f16).T
        w[64] = (np.asarray(i[f"bih_{s}"], np.float32)
                 + np.asarray(i[f"bhh_{s}"], np.float32)).astype(f16)
        return w
    wih = np.stack([_wih_aug("l"), _wih_aug("r"), _wih_aug("d")])
    w3t = np.ascontiguousarray(
        i["W3"].astype(f16).T.reshape(16, P, 10).transpose(1, 0, 2))
    b3t = i["b3"].astype(f16).reshape(1, 10)

    in_maps = []
    for c in range(NCORES):
        bs = slice(c * B, (c + 1) * B)
        xt = np.ascontiguousarray(
            i["x"][bs].astype(f16).T.reshape(KIN, P, B).transpose(1, 0, 2))
        h0t = np.stack([_hidT(i["h0"][j, bs].astype(f16)) for j in range(4)])
        c0t = np.stack([_hidT(i["c0"][j, bs].astype(f32)) for j in range(4)])
        in_maps.append({
            "xt": xt, "w1t": w1t, "b1t": b1t, "whh": whh, "wih": wih,
            "h0t": h0t, "c0t": c0t, "w3t": w3t, "b3t": b3t,
        })
    return in_maps


def kernel(**inputs) -> np.ndarray:
    nc = build()
    in_maps = _prep(inputs)
    res = bass_utils.run_bass_kernel_spmd(
        nc, in_maps, core_ids=list(range(NCORES)), trace=False)
    return np.concatenate(
        [res.results[c]["out"] for c in range(NCORES)], axis=0)
